# revision 9
# baseline (speedup 1.0000x reference)
"""Multi-head attention on 8 TRN2 NeuronCores (Bass/Tile).

Sharding: core c handles batch b = c//2 and query-half h = c%2 (1024 query
tokens), all 16 heads. K/V projections are per-batch and duplicated across
the two cores sharing a batch, so no cross-core communication is needed;
the host splits inputs and concatenates outputs.

Design notes:
- Every on-chip tensor keeps tokens on the free axis and embed/head_dim on
  partitions. Q/K projections then directly produce the Q^T/K^T tiles the
  energy matmul wants, and the output projection directly produces Y^T
  (transposed back on the host).
- Keys are compacted on the host using the 0/1 key mask (masked keys dropped,
  padded to a multiple of 128). Pad positions are killed inside the softmax
  by a -1e9 per-partition bias folded into the Exp activation. This halves
  the attention and K/V projection work for a ~half-zero mask.
- Energy is computed transposed ([key, query] tiles) so the softmax
  normalizer can ride the attention@V matmul: each head's V tile carries an
  extra ones column, so the AV matmul (M=65) yields 64 output rows plus the
  softmax denominator row. Normalization is reciprocal + gpsimd
  partition-broadcast + one elementwise multiply per head.
- Matmul operand tensors are typed float32r (TF32-like), streaming at 1
  cycle/row vs fp32's 4, with fp32 PSUM accumulation.
"""

import sys

sys.path.insert(0, "/opt/trn_rl_repo")

from contextlib import ExitStack

import numpy as np

import concourse.bass as bass  # noqa: F401  (engine types via nc)
import concourse.tile as tile
from concourse import bacc, mybir
from concourse.bass_utils import run_bass_kernel_spmd

E = 1024          # embed dim
HEADS = 16
HD = 64           # head dim
B = 4
S = 2048
NCORES = 8
Q = (B * S) // NCORES  # query tokens per core
EC = E // 128     # embed chunks of 128
F32 = mybir.dt.float32
F32R = mybir.dt.float32r

USE_F32R = True


def _nchunks(total, pref=512, minsz=256):
    """Split `total` (multiple of 128, >=256) into chunks in [minsz, pref]."""
    out, rem = [], total
    while rem > 0:
        c = min(pref, rem)
        if rem - c != 0 and rem - c < minsz:
            c = rem - minsz
        out.append(c)
        rem -= c
    return out


def _starts(chunks):
    s, out = 0, []
    for c in chunks:
        out.append((s, c))
        s += c
    return out


def build_program(Kpad):
    """Build the per-core Bass program (identical on all 8 cores)."""
    KTn = Kpad // 128
    nc = bacc.Bacc("TRN2", target_bir_lowering=False, debug=False,
                   num_devices=NCORES)

    qT = nc.dram_tensor("qT", [E, Q], F32R, kind="ExternalInput").ap()
    kT = nc.dram_tensor("kT", [E, Kpad], F32R, kind="ExternalInput").ap()
    vT = nc.dram_tensor("vT", [E, Kpad], F32R, kind="ExternalInput").ap()
    wqT = nc.dram_tensor("wqT", [E, E], F32R, kind="ExternalInput").ap()
    wkT = nc.dram_tensor("wkT", [E, E], F32R, kind="ExternalInput").ap()
    wvT = nc.dram_tensor("wvT", [E, E], F32R, kind="ExternalInput").ap()
    woT = nc.dram_tensor("woT", [E, E], F32R, kind="ExternalInput").ap()
    bq2 = nc.dram_tensor("bq2", [128, EC], F32, kind="ExternalInput").ap()
    bk2 = nc.dram_tensor("bk2", [128, EC], F32, kind="ExternalInput").ap()
    bo2 = nc.dram_tensor("bo2", [128, EC], F32, kind="ExternalInput").ap()
    bv2 = nc.dram_tensor("bv2", [1, E], F32R, kind="ExternalInput").ap()
    mb = nc.dram_tensor("mb", [128, KTn], F32, kind="ExternalInput").ap()
    onesd = nc.dram_tensor("onesd", [128, 128], F32R, kind="ExternalInput").ap()
    yT = nc.dram_tensor("yT", [E, Q], F32, kind="ExternalOutput").ap()

    q_chunks = _starts(_nchunks(Q))
    k_chunks = _starts(_nchunks(Kpad))

    with tile.TileContext(nc) as tc, ExitStack() as ctx:
        inp = ctx.enter_context(tc.tile_pool(name="inp", bufs=8))
        wgt = ctx.enter_context(tc.tile_pool(name="wgt", bufs=8))
        big = ctx.enter_context(tc.tile_pool(name="big", bufs=1))
        ptp = ctx.enter_context(tc.tile_pool(name="ptp", bufs=2))
        nrm = ctx.enter_context(tc.tile_pool(name="nrm", bufs=4))
        sml = ctx.enter_context(tc.tile_pool(name="sml", bufs=1))

        # ---- small constants -------------------------------------------
        bq_s = sml.tile([128, EC], F32, name="bq_s")
        nc.sync.dma_start(bq_s[:], bq2[:])
        bk_s = sml.tile([128, EC], F32, name="bk_s")
        nc.sync.dma_start(bk_s[:], bk2[:])
        bo_s = sml.tile([128, EC], F32, name="bo_s")
        nc.sync.dma_start(bo_s[:], bo2[:])
        bv_s = sml.tile([1, E], F32R, name="bv_s")
        nc.sync.dma_start(bv_s[:], bv2[:])
        mb_s = sml.tile([128, KTn], F32, name="mb_s")
        nc.sync.dma_start(mb_s[:], mb[:])
        ones_s = sml.tile([128, 128], F32R, name="ones_s")
        nc.sync.dma_start(ones_s[:], onesd[:])
        ones1 = ones_s

        def proj(dst_tiles, x_dram, w_dram, chunks, bias_s, ps_pool, xname):
            """dst[m] = (W @ X^T + b) tiles: [128, n_total] each, m in 0..7.

            Each n-chunk accumulates in its own PSUM bank (matmul output
            cannot cross a 512-float bank boundary).
            """
            xc = []
            ntot = chunks[-1][0] + chunks[-1][1]
            for k in range(EC):
                t = inp.tile([128, ntot], F32R, tag="ic", name=f"{xname}{k}")
                nc.sync.dma_start(t[:], x_dram[k * 128:(k + 1) * 128, :])
                xc.append(t)
            wc = []
            for k in range(EC):
                t = wgt.tile([128, E], F32R, tag="wc", name=f"w{xname}{k}")
                nc.sync.dma_start(t[:], w_dram[k * 128:(k + 1) * 128, :])
                wc.append(t)
            for m in range(EC):
                ps = ps_pool.tile([128, 512 * len(chunks)], F32, tag="mm",
                                  name=f"ps{xname}{m}")
                for i, (n0, nn) in enumerate(chunks):
                    for k in range(EC):
                        nc.tensor.matmul(
                            ps[:, i * 512:i * 512 + nn],
                            wc[k][:, m * 128:(m + 1) * 128],
                            xc[k][:, n0:n0 + nn],
                            start=(k == 0), stop=(k == EC - 1))
                for i, (n0, nn) in enumerate(chunks):
                    nc.vector.tensor_scalar_add(
                        dst_tiles[m][:, n0:n0 + nn],
                        ps[:, i * 512:i * 512 + nn], bias_s[:, m:m + 1])

        # ---- phases A/B: Q and K projections ---------------------------
        QTs = [big.tile([128, Q], F32R, name=f"QT{m}", tag=f"QT{m}")
               for m in range(EC)]
        KTs = [big.tile([128, Kpad], F32R, name=f"KT{m}", tag=f"KT{m}")
               for m in range(EC)]
        VVs = [big.tile([128, HEADS * 65], F32R, name=f"VV{t}", tag=f"VV{t}")
               for t in range(KTn)]

        with tc.tile_pool(name="psA", bufs=2, space="PSUM") as psA:
            proj(QTs, qT, wqT, q_chunks, bq_s, psA, "q")
            proj(KTs, kT, wkT, k_chunks, bk_s, psA, "k")

            # ---- phase C: V projection in [token, head_dim] layout -----
            vc = []
            for k in range(EC):
                t = inp.tile([128, Kpad], F32R, tag="ic", name=f"v{k}")
                nc.sync.dma_start(t[:], vT[k * 128:(k + 1) * 128, :])
                vc.append(t)
            wvc = []
            for k in range(EC):
                t = wgt.tile([128, E], F32R, tag="wc", name=f"wv{k}")
                nc.sync.dma_start(t[:], wvT[k * 128:(k + 1) * 128, :])
                wvc.append(t)
            for t in range(KTn):
                ps = psA.tile([128, E], F32, tag="mm", name=f"psv{t}")
                for n0 in (0, 512):
                    for k in range(EC):
                        nc.tensor.matmul(
                            ps[:, n0:n0 + 512],
                            vc[k][:, t * 128:(t + 1) * 128],
                            wvc[k][:, n0:n0 + 512],
                            start=(k == 0), stop=False)
                    # bias via K=1 ones row: V += 1 * bv
                    nc.tensor.matmul(
                        ps[:, n0:n0 + 512],
                        ones1[0:1, 0:128],
                        bv_s[0:1, n0:n0 + 512],
                        start=False, stop=True)
                vv3 = VVs[t][:].rearrange("p (h e) -> p h e", e=65)
                ps3 = ps[:].rearrange("p (h d) -> p h d", d=64)
                nc.vector.tensor_copy(vv3[:, :, 0:64], ps3[:])
                nc.vector.tensor_copy(
                    vv3[:, :, 64:65],
                    ones_s[:, 0:16].rearrange("p (a b) -> p a b", b=1))

        # ---- phase D: attention per head pair --------------------------
        with tc.tile_pool(name="psD", bufs=2, space="PSUM") as psD:
            for j in range(EC):  # head pair j -> heads 2j (rows 0:64), 2j+1
                po = []
                for hh in (0, 1):
                    po.append(psD.tile([65, Q], F32, tag="po",
                                       name=f"po{j}_{hh}"))
                for kt in range(KTn):
                    pe = []
                    for hh in (0, 1):
                        pe.append(psD.tile([128, Q], F32, tag="pe",
                                           name=f"pe{j}_{kt}_{hh}"))
                    for n0 in (0, 512):
                        for hh in (0, 1):  # adjacent => row-group overlap
                            off = hh * 64
                            nc.tensor.matmul(
                                pe[hh][:, n0:n0 + 512],
                                KTs[j][off:off + 64,
                                          kt * 128:(kt + 1) * 128],
                                QTs[j][off:off + 64, n0:n0 + 512])
                    pt = []
                    for hh in (0, 1):
                        t = ptp.tile([128, Q], F32R, tag="pt",
                                     name=f"pt{j}_{kt}_{hh}")
                        nc.scalar.activation(
                            t[:], pe[hh][:], mybir.ActivationFunctionType.Exp,
                            bias=mb_s[:, kt:kt + 1], scale=0.125)
                        pt.append(t)
                    for n0 in (0, 512):
                        for hh in (0, 1):
                            h = 2 * j + hh
                            nc.tensor.matmul(
                                po[hh][0:65, n0:n0 + 512],
                                VVs[kt][:, h * 65:(h + 1) * 65],
                                pt[hh][:, n0:n0 + 512],
                                start=(kt == 0), stop=(kt == KTn - 1))
                # normalize: out rows = po[0:64] * (1 / po[64]) broadcast.
                # Broadcast along partitions via a K=1 ones matmul (PE).
                for hh in (0, 1):
                    rc = nrm.tile([65, Q], F32R, tag="s", name=f"rc{j}_{hh}")
                    with nc.allow_low_precision(
                            reason="f32r recip feeds f32r matmul broadcast"):
                        nc.vector.reciprocal(rc[64:65, :], po[hh][64:65, :])
                    bc_ps = psD.tile([64, Q], F32, tag="pe",
                                     name=f"bp{j}_{hh}")
                    for n0 in (0, 512):
                        nc.tensor.matmul(bc_ps[0:64, n0:n0 + 512],
                                         ones_s[64:65, 0:64],
                                         rc[64:65, n0:n0 + 512],
                                         start=True, stop=True)
                    bc = nrm.tile([64, Q], F32, tag="s", name=f"bc{j}_{hh}")
                    nc.vector.tensor_copy(bc[:], bc_ps[:])
                    if hh == 0:
                        nc.vector.tensor_mul(QTs[j][0:64, :], po[hh][0:64, :],
                                             bc[0:64, :])
                    else:
                        tmp = nrm.tile([64, Q], F32R, tag="s", name=f"tm{j}")
                        nc.vector.tensor_mul(tmp[:], po[hh][0:64, :],
                                             bc[0:64, :])
                        # partition shift 0:64 -> 64:128 via SBUF-SBUF DMA
                        nc.sync.dma_start(QTs[j][64:128, :], tmp[:])

        # ---- phase E: output projection Y^T = Wo @ O^T + bo ------------
        with tc.tile_pool(name="psE", bufs=2, space="PSUM") as psE:
            woc = []
            for k in range(EC):
                t = wgt.tile([128, E], F32R, tag="wc", name=f"wo{k}")
                nc.sync.dma_start(t[:], woT[k * 128:(k + 1) * 128, :])
                woc.append(t)
            for m in range(EC):
                ps = psE.tile([128, Q], F32, tag="mm2", name=f"psy{m}")
                for n0, nn in q_chunks:
                    for k in range(EC):
                        nc.tensor.matmul(
                            ps[:, n0:n0 + nn],
                            woc[k][:, m * 128:(m + 1) * 128],
                            QTs[k][:, n0:n0 + nn],
                            start=(k == 0), stop=(k == EC - 1))
                yt = nrm.tile([128, Q], F32, tag="s", name=f"yt{m}")
                nc.vector.tensor_scalar_add(yt[:], ps[:], bo_s[:, m:m + 1])
                nc.sync.dma_start(yT[m * 128:(m + 1) * 128, :], yt[:])

    nc.compile()
    return nc


_ONES128 = np.ones((128, 128), np.float32)

_PROG_CACHE = {}


def _get_program(Kpad):
    key = (Kpad, USE_F32R)
    if key not in _PROG_CACHE:
        _PROG_CACHE[key] = build_program(Kpad)
    return _PROG_CACHE[key]


def prepare_inputs(query, keys, values, mask, Wq, bq, Wk, bk, Wv, bv, Wo, bo):
    """Host-side sharding/layout prep. Returns (Kpad, in_maps)."""
    f32 = np.float32
    query = np.asarray(query, f32)
    keys = np.asarray(keys, f32)
    values = np.asarray(values, f32)
    mask = np.asarray(mask)

    idxs = [np.nonzero(mask[b] != 0)[0] for b in range(B)]
    nmax = max(len(i) for i in idxs)
    Kpad = max(256, ((max(nmax, 1) + 127) // 128) * 128)
    KTn = Kpad // 128

    kTb = np.zeros((B, E, Kpad), f32)
    vTb = np.zeros((B, E, Kpad), f32)
    mbb = np.full((B, Kpad), -1e9, f32)
    for b in range(B):
        n = len(idxs[b])
        kTb[b, :, :n] = keys[b][idxs[b]].T
        vTb[b, :, :n] = values[b][idxs[b]].T
        mbb[b, :n] = 0.0
    mb2 = np.ascontiguousarray(mbb.reshape(B, KTn, 128).transpose(0, 2, 1))

    WqT = np.ascontiguousarray(np.asarray(Wq, f32).T)
    WkT = np.ascontiguousarray(np.asarray(Wk, f32).T)
    WvT = np.ascontiguousarray(np.asarray(Wv, f32).T)
    WoT = np.ascontiguousarray(np.asarray(Wo, f32).T)
    bq2 = np.ascontiguousarray(np.asarray(bq, f32).reshape(EC, 128).T)
    bk2 = np.ascontiguousarray(np.asarray(bk, f32).reshape(EC, 128).T)
    bo2 = np.ascontiguousarray(np.asarray(bo, f32).reshape(EC, 128).T)
    bv2 = np.ascontiguousarray(np.asarray(bv, f32).reshape(1, E))

    in_maps = []
    for c in range(NCORES):
        b, h = c // 2, c % 2
        in_maps.append(dict(
            qT=np.ascontiguousarray(query[b, h * Q:(h + 1) * Q, :].T),
            kT=kTb[b], vT=vTb[b], mb=mb2[b],
            wqT=WqT, wkT=WkT, wvT=WvT, woT=WoT,
            bq2=bq2, bk2=bk2, bo2=bo2, bv2=bv2,
            onesd=_ONES128,
        ))
    return Kpad, in_maps


def kernel(query, keys, values, mask, Wq, bq, Wk, bk, Wv, bv, Wo, bo):
    Kpad, in_maps = prepare_inputs(query, keys, values, mask,
                                   Wq, bq, Wk, bk, Wv, bv, Wo, bo)
    nc = _get_program(Kpad)
    res = run_bass_kernel_spmd(nc, in_maps, list(range(NCORES)))
    out = np.empty((B, S, E), np.float32)
    for c in range(NCORES):
        b, h = c // 2, c % 2
        out[b, h * Q:(h + 1) * Q, :] = res.results[c]["yT"].T
    return out
